# revision 6
# baseline (speedup 1.0000x reference)
"""Trainium2 Bass kernel for NeRF-style volume rendering (PiggyBackBase.distill).

Computes, per ray (row) over K=512 samples:
    alpha   = 1 - exp(-sigma * dists * 25.0)
    T       = exclusive cumprod of (1 - alpha + eps)     (transmittance)
    weights = alpha * T
    rgb_map = clip(sum_k w * rgb + bg, 0, 1)             (white background)
    depth   = sum_k w * z + bg * far
    bg      = T[K] (full product)

Sharding: data-parallel over the 32768 rays across 8 NeuronCores
(4096 rays/core); the cumprod scan runs along the free axis via the
DVE TensorTensorScan instruction, reductions via fused
tensor_tensor_reduce. No cross-core communication.
"""

import numpy as np

import concourse.bacc as bacc
import concourse.tile as tile
from concourse import mybir

N_RAYS = 32768
K = 512
NCORES = 8
NPER = N_RAYS // NCORES  # rays per core
P = 128                  # SBUF partitions (rays per block)
DIST_SCALE = 25.0
EPS = 1e-10
F32 = mybir.dt.float32

Alu = mybir.AluOpType
Act = mybir.ActivationFunctionType


def build_program(nper: int = NPER, blocks_per_iter: int = 4):
    """Build the single-core Bass program operating on an [nper, K] shard."""
    B = blocks_per_iter
    NB = nper // P           # ray-blocks of 128
    NIT = NB // B            # macro iterations
    assert NB * P == nper and NIT * B == NB

    nc = bacc.Bacc("TRN2", target_bir_lowering=False, debug=False)

    sigma = nc.dram_tensor("sigma", [nper, K], F32, kind="ExternalInput").ap()
    rgb = nc.dram_tensor("rgb", [nper, K, 3], F32, kind="ExternalInput").ap()
    dists = nc.dram_tensor("dists", [nper, K], F32, kind="ExternalInput").ap()
    z_vals = nc.dram_tensor("z_vals", [nper, K], F32, kind="ExternalInput").ap()
    rays_far = nc.dram_tensor("rays_far", [nper], F32, kind="ExternalInput").ap()

    rgb_map = nc.dram_tensor("rgb_map", [nper, 3], F32, kind="ExternalOutput").ap()
    depth_map = nc.dram_tensor("depth_map", [nper], F32, kind="ExternalOutput").ap()
    alpha_o = nc.dram_tensor("alpha", [nper, K], F32, kind="ExternalOutput").ap()
    weights_o = nc.dram_tensor("weights", [nper, K], F32, kind="ExternalOutput").ap()
    bg_o = nc.dram_tensor("bg_weight", [nper, 1], F32, kind="ExternalOutput").ap()

    # [ray, ...] -> [partition, block, ...]; ray = block*128 + partition
    sig_r = sigma.rearrange("(n p) k -> p n k", p=P)
    rgb_r = rgb.rearrange("(n p) k c -> p n k c", p=P)
    dst_r = dists.rearrange("(n p) k -> p n k", p=P)
    z_r = z_vals.rearrange("(n p) k -> p n k", p=P)
    far_r = rays_far.rearrange("(n p) -> p n", p=P)
    rgbm_r = rgb_map.rearrange("(n p) c -> p n c", p=P)
    depth_r = depth_map.rearrange("(n p) -> p n", p=P)
    al_r = alpha_o.rearrange("(n p) k -> p n k", p=P)
    w_r = weights_o.rearrange("(n p) k -> p n k", p=P)
    bg_r = bg_o.rearrange("(n p) c -> p n c", p=P)

    with tile.TileContext(nc) as tc:
        with tc.tile_pool(name="big", bufs=2) as pool, \
             tc.tile_pool(name="outs", bufs=1) as spool:
            # Small per-ray outputs accumulated across the whole shard, one
            # DMA each at the end.
            depth_t = spool.tile([P, NB], F32)
            rgbm_t = spool.tile([P, NB, 3], F32)
            bg_t = spool.tile([P, NB, 1], F32)

            for i in range(NIT):
                sl = slice(i * B, (i + 1) * B)

                sig_t = pool.tile([P, B, K], F32)
                nc.sync.dma_start(out=sig_t, in_=sig_r[:, sl, :])
                dst_t = pool.tile([P, B, K], F32)
                nc.sync.dma_start(out=dst_t, in_=dst_r[:, sl, :])
                z_t = pool.tile([P, B, K], F32)
                nc.sync.dma_start(out=z_t, in_=z_r[:, sl, :])
                rgb_t = pool.tile([P, B, K, 3], F32)
                nc.sync.dma_start(out=rgb_t, in_=rgb_r[:, sl, :, :])
                far_t = pool.tile([P, B], F32)
                nc.sync.dma_start(out=far_t, in_=far_r[:, sl])

                # y = sigma * dists (in place into sig_t)
                nc.vector.tensor_mul(out=sig_t, in0=sig_t, in1=dst_t)

                # em = [1, e_0 .. e_K-1] per block, e = exp(-25*y) + eps
                em_t = pool.tile([P, B, K + 1], F32)
                nc.vector.memset(em_t[:, :, 0:1], 1.0)
                nc.scalar.activation(
                    out=em_t[:, :, 1 : K + 1],
                    in_=sig_t,
                    func=Act.Exp,
                    scale=-DIST_SCALE,
                )
                # alpha = 1 - e  (before the eps nudge, matching reference)
                al_t = pool.tile([P, B, K], F32)
                nc.vector.tensor_scalar(
                    out=al_t,
                    in0=em_t[:, :, 1 : K + 1],
                    scalar1=-1.0,
                    scalar2=1.0,
                    op0=Alu.mult,
                    op1=Alu.add,
                )
                # e += eps so the cumprod matches cumprod(1 - alpha + eps)
                nc.scalar.activation(
                    out=em_t[:, :, 1 : K + 1],
                    in_=em_t[:, :, 1 : K + 1],
                    func=Act.Copy,
                    bias=EPS,
                )

                # exclusive cumprod via hardware prefix scan (per block)
                T_t = pool.tile([P, B, K], F32)
                for b in range(B):
                    nc.vector.tensor_tensor_scan(
                        out=T_t[:, b, :],
                        data0=em_t[:, b, 0:K],
                        data1=em_t[:, b, 0:K],
                        initial=1.0,
                        op0=Alu.mult,
                        op1=Alu.bypass,
                    )

                # weights = alpha * T
                w_t = pool.tile([P, B, K], F32)
                nc.vector.tensor_mul(out=w_t, in0=al_t, in1=T_t)

                # bg = T[:, K-1] * em[:, K]  (full product incl. last sample)
                nc.vector.tensor_mul(
                    out=bg_t[:, sl, 0],
                    in0=T_t[:, :, K - 1],
                    in1=em_t[:, :, K],
                )

                # depth0 = sum_k w * z   (fused multiply+reduce; dst_t is scratch)
                dsum_t = pool.tile([P, B], F32)
                for b in range(B):
                    nc.vector.scalar_tensor_tensor(
                        out=dst_t[:, b, :],
                        in0=w_t[:, b, :],
                        scalar=1.0,
                        in1=z_t[:, b, :],
                        op0=Alu.bypass,
                        op1=Alu.mult,
                        accum_out=dsum_t[:, b : b + 1],
                    )
                # depth = depth0 + bg * far   (1 - acc == bg up to fp32 eps)
                tmp_t = pool.tile([P, B], F32)
                nc.vector.tensor_mul(out=tmp_t, in0=bg_t[:, sl, 0], in1=far_t)
                nc.vector.tensor_add(out=depth_t[:, sl], in0=dsum_t, in1=tmp_t)

                # rgb sums per channel
                rsum_t = pool.tile([P, B, 3], F32)
                for c in range(3):
                    for b in range(B):
                        nc.vector.scalar_tensor_tensor(
                            out=dst_t[:, b, :],
                            in0=w_t[:, b, :],
                            scalar=1.0,
                            in1=rgb_t[:, b, :, c],
                            op0=Alu.bypass,
                            op1=Alu.mult,
                            accum_out=rsum_t[:, b, c : c + 1],
                        )
                # rgb_map = clip(rsum + bg, 0, 1)
                for b in range(B):
                    nc.vector.tensor_scalar(
                        out=rgbm_t[:, i * B + b, :],
                        in0=rsum_t[:, b, :],
                        scalar1=bg_t[:, i * B + b, :],
                        scalar2=1.0,
                        op0=Alu.add,
                        op1=Alu.min,
                    )
                nc.vector.tensor_scalar_max(
                    out=rgbm_t[:, sl, :], in0=rgbm_t[:, sl, :], scalar1=0.0
                )

                nc.sync.dma_start(out=al_r[:, sl, :], in_=al_t)
                nc.sync.dma_start(out=w_r[:, sl, :], in_=w_t)

            nc.sync.dma_start(out=depth_r, in_=depth_t)
            nc.sync.dma_start(out=rgbm_r, in_=rgbm_t)
            nc.sync.dma_start(out=bg_r, in_=bg_t)

    nc.finalize()
    return nc


_PROG = None


def _get_prog():
    global _PROG
    if _PROG is None:
        _PROG = build_program()
    return _PROG


def kernel(sigma, rgb, dists, z_vals, rays_far):
    from concourse.bass_utils import run_bass_kernel_spmd

    nc = _get_prog()
    in_maps = []
    for c in range(NCORES):
        sl = slice(c * NPER, (c + 1) * NPER)
        in_maps.append(
            {
                "sigma": np.ascontiguousarray(sigma[sl], dtype=np.float32),
                "rgb": np.ascontiguousarray(rgb[sl], dtype=np.float32),
                "dists": np.ascontiguousarray(dists[sl], dtype=np.float32),
                "z_vals": np.ascontiguousarray(z_vals[sl], dtype=np.float32),
                "rays_far": np.ascontiguousarray(rays_far[sl], dtype=np.float32),
            }
        )
    res = run_bass_kernel_spmd(nc, in_maps, core_ids=list(range(NCORES)))
    outs = res.results
    rgb_map = np.concatenate([outs[c]["rgb_map"] for c in range(NCORES)], axis=0)
    depth_map = np.concatenate([outs[c]["depth_map"] for c in range(NCORES)], axis=0)
    alpha = np.concatenate([outs[c]["alpha"] for c in range(NCORES)], axis=0)
    weights = np.concatenate([outs[c]["weights"] for c in range(NCORES)], axis=0)
    bg_weight = np.concatenate([outs[c]["bg_weight"] for c in range(NCORES)], axis=0)
    return rgb_map, depth_map, alpha, weights, bg_weight


# revision 9
# speedup vs baseline: 13.0217x; 13.0217x over previous
"""Trainium2 Bass kernel for NeRF-style volume rendering (PiggyBackBase.distill).

Computes, per ray (row) over K=512 samples:
    alpha   = 1 - exp(-sigma * dists * 25.0)
    T       = exclusive cumprod of (1 - alpha + eps)     (transmittance)
    weights = alpha * T
    rgb_map = clip(sum_k w * rgb + bg, 0, 1)             (white background)
    depth   = sum_k w * z + bg * far
    bg      = T[K] (full product)

Sharding: data-parallel over the 32768 rays across 8 NeuronCores
(4096 rays/core); the cumprod scan runs along the free axis via the
DVE TensorTensorScan instruction, reductions via fused
tensor_tensor_reduce. No cross-core communication.
"""

import numpy as np

import concourse.bacc as bacc
import concourse.tile as tile
from concourse import mybir

N_RAYS = 32768
K = 512
NCORES = 8
NPER = N_RAYS // NCORES  # rays per core
P = 128                  # SBUF partitions (rays per block)
DIST_SCALE = 25.0
EPS = 1e-10
F32 = mybir.dt.float32

Alu = mybir.AluOpType
Act = mybir.ActivationFunctionType


def build_program(
    nper: int = NPER,
    blocks_per_iter: int = 4,
    eng_y: str = "vector",
    eng_alpha: str = "vector",
    with_eps: bool = True,
    compute: bool = True,
):
    """Build the single-core Bass program operating on an [nper, K] shard."""
    B = blocks_per_iter
    NB = nper // P           # ray-blocks of 128
    NIT = NB // B            # macro iterations
    assert NB * P == nper and NIT * B == NB

    nc = bacc.Bacc("TRN2", target_bir_lowering=False, debug=False)

    sigma = nc.dram_tensor("sigma", [nper, K], F32, kind="ExternalInput").ap()
    rgb = nc.dram_tensor("rgb", [nper, K, 3], F32, kind="ExternalInput").ap()
    dists = nc.dram_tensor("dists", [nper, K], F32, kind="ExternalInput").ap()
    z_vals = nc.dram_tensor("z_vals", [nper, K], F32, kind="ExternalInput").ap()
    rays_far = nc.dram_tensor("rays_far", [nper], F32, kind="ExternalInput").ap()

    rgb_map = nc.dram_tensor("rgb_map", [nper, 3], F32, kind="ExternalOutput").ap()
    depth_map = nc.dram_tensor("depth_map", [nper], F32, kind="ExternalOutput").ap()
    alpha_o = nc.dram_tensor("alpha", [nper, K], F32, kind="ExternalOutput").ap()
    weights_o = nc.dram_tensor("weights", [nper, K], F32, kind="ExternalOutput").ap()
    bg_o = nc.dram_tensor("bg_weight", [nper, 1], F32, kind="ExternalOutput").ap()

    # [ray, ...] -> [partition, block, ...]; ray = block*128 + partition
    sig_r = sigma.rearrange("(n p) k -> p n k", p=P)
    rgb_r = rgb.rearrange("(n p) k c -> p n k c", p=P)
    dst_r = dists.rearrange("(n p) k -> p n k", p=P)
    z_r = z_vals.rearrange("(n p) k -> p n k", p=P)
    far_r = rays_far.rearrange("(n p) -> p n", p=P)
    rgbm_r = rgb_map.rearrange("(n p) c -> p n c", p=P)
    depth_r = depth_map.rearrange("(n p) -> p n", p=P)
    al_r = alpha_o.rearrange("(n p) k -> p n k", p=P)
    w_r = weights_o.rearrange("(n p) k -> p n k", p=P)
    bg_r = bg_o.rearrange("(n p) c -> p n c", p=P)

    with tile.TileContext(nc) as tc:
        with tc.tile_pool(name="big", bufs=2) as pool, \
             tc.tile_pool(name="outs", bufs=1) as spool:
            # Small per-ray outputs accumulated across the whole shard, one
            # DMA each at the end.
            depth_t = spool.tile([P, NB], F32)
            rgbm_t = spool.tile([P, NB, 3], F32)
            bg_t = spool.tile([P, NB, 1], F32)

            for i in range(NIT):
                sl = slice(i * B, (i + 1) * B)

                sig_t = pool.tile([P, B, K], F32)
                nc.sync.dma_start(out=sig_t, in_=sig_r[:, sl, :])
                dst_t = pool.tile([P, B, K], F32)
                nc.sync.dma_start(out=dst_t, in_=dst_r[:, sl, :])
                z_t = pool.tile([P, B, K], F32)
                nc.sync.dma_start(out=z_t, in_=z_r[:, sl, :])
                rgb_t = pool.tile([P, B, K, 3], F32)
                nc.sync.dma_start(out=rgb_t, in_=rgb_r[:, sl, :, :])
                far_t = pool.tile([P, B], F32)
                nc.sync.dma_start(out=far_t, in_=far_r[:, sl])

                if not compute:
                    # DMA-floor diagnostic: skip all compute, emit dummy
                    # writes so output tiles exist.
                    al_t = pool.tile([P, B, K], F32)
                    w_t = pool.tile([P, B, K], F32)
                    nc.vector.tensor_copy(out=al_t, in_=sig_t)
                    nc.vector.tensor_copy(out=w_t, in_=z_t)
                    nc.vector.tensor_copy(out=depth_t[:, sl], in_=far_t)
                    nc.vector.tensor_copy(out=bg_t[:, sl, 0], in_=far_t)
                    nc.vector.tensor_copy(
                        out=rgbm_t[:, sl, :], in_=rgb_t[:, :, 0, :]
                    )
                    nc.sync.dma_start(out=al_r[:, sl, :], in_=al_t)
                    nc.sync.dma_start(out=w_r[:, sl, :], in_=w_t)
                    continue

                eng_y_ns = getattr(nc, eng_y)
                eng_alpha_ns = getattr(nc, eng_alpha)
                # y = sigma * dists (in place into sig_t)
                eng_y_ns.tensor_mul(out=sig_t, in0=sig_t, in1=dst_t)

                # em = [1, e_0 .. e_K-1] per block, e = exp(-25*y) + eps
                em_t = pool.tile([P, B, K + 1], F32)
                nc.vector.memset(em_t[:, :, 0:1], 1.0)
                nc.scalar.activation(
                    out=em_t[:, :, 1 : K + 1],
                    in_=sig_t,
                    func=Act.Exp,
                    scale=-DIST_SCALE,
                )
                # alpha = 1 - e  (before the eps nudge, matching reference)
                al_t = pool.tile([P, B, K], F32)
                eng_alpha_ns.tensor_scalar(
                    out=al_t,
                    in0=em_t[:, :, 1 : K + 1],
                    scalar1=-1.0,
                    scalar2=1.0,
                    op0=Alu.mult,
                    op1=Alu.add,
                )
                if with_eps:
                    # e += eps so the cumprod matches cumprod(1 - alpha + eps)
                    nc.scalar.activation(
                        out=em_t[:, :, 1 : K + 1],
                        in_=em_t[:, :, 1 : K + 1],
                        func=Act.Copy,
                        bias=EPS,
                    )

                # exclusive cumprod via hardware prefix scan (per block)
                T_t = pool.tile([P, B, K], F32)
                for b in range(B):
                    nc.vector.tensor_tensor_scan(
                        out=T_t[:, b, :],
                        data0=em_t[:, b, 0:K],
                        data1=em_t[:, b, 0:K],
                        initial=1.0,
                        op0=Alu.mult,
                        op1=Alu.bypass,
                    )

                # weights = alpha * T
                w_t = pool.tile([P, B, K], F32)
                nc.vector.tensor_mul(out=w_t, in0=al_t, in1=T_t)

                # bg = T[:, K-1] * em[:, K]  (full product incl. last sample)
                nc.vector.tensor_mul(
                    out=bg_t[:, sl, 0],
                    in0=T_t[:, :, K - 1],
                    in1=em_t[:, :, K],
                )

                # depth0 = sum_k w * z   (fused multiply+reduce; dst_t is scratch)
                dsum_t = pool.tile([P, B], F32)
                for b in range(B):
                    nc.vector.scalar_tensor_tensor(
                        out=dst_t[:, b, :],
                        in0=w_t[:, b, :],
                        scalar=1.0,
                        in1=z_t[:, b, :],
                        op0=Alu.bypass,
                        op1=Alu.mult,
                        accum_out=dsum_t[:, b : b + 1],
                    )
                # depth = depth0 + bg * far   (1 - acc == bg up to fp32 eps)
                tmp_t = pool.tile([P, B], F32)
                nc.vector.tensor_mul(out=tmp_t, in0=bg_t[:, sl, 0], in1=far_t)
                nc.vector.tensor_add(out=depth_t[:, sl], in0=dsum_t, in1=tmp_t)

                # rgb sums per channel
                rsum_t = pool.tile([P, B, 3], F32)
                for c in range(3):
                    for b in range(B):
                        nc.vector.scalar_tensor_tensor(
                            out=dst_t[:, b, :],
                            in0=w_t[:, b, :],
                            scalar=1.0,
                            in1=rgb_t[:, b, :, c],
                            op0=Alu.bypass,
                            op1=Alu.mult,
                            accum_out=rsum_t[:, b, c : c + 1],
                        )
                # rgb_map = clip(rsum + bg, 0, 1)
                for b in range(B):
                    nc.vector.tensor_scalar(
                        out=rgbm_t[:, i * B + b, :],
                        in0=rsum_t[:, b, :],
                        scalar1=bg_t[:, i * B + b, :],
                        scalar2=1.0,
                        op0=Alu.add,
                        op1=Alu.min,
                    )
                nc.vector.tensor_scalar_max(
                    out=rgbm_t[:, sl, :], in0=rgbm_t[:, sl, :], scalar1=0.0
                )

                nc.sync.dma_start(out=al_r[:, sl, :], in_=al_t)
                nc.sync.dma_start(out=w_r[:, sl, :], in_=w_t)

            nc.sync.dma_start(out=depth_r, in_=depth_t)
            nc.sync.dma_start(out=rgbm_r, in_=rgbm_t)
            nc.sync.dma_start(out=bg_r, in_=bg_t)

    nc.finalize()
    return nc


_PROG = None


def _get_prog():
    global _PROG
    if _PROG is None:
        _PROG = build_program()
    return _PROG


def kernel(sigma, rgb, dists, z_vals, rays_far):
    from concourse.bass_utils import run_bass_kernel_spmd

    nc = _get_prog()
    in_maps = []
    for c in range(NCORES):
        sl = slice(c * NPER, (c + 1) * NPER)
        in_maps.append(
            {
                "sigma": np.ascontiguousarray(sigma[sl], dtype=np.float32),
                "rgb": np.ascontiguousarray(rgb[sl], dtype=np.float32),
                "dists": np.ascontiguousarray(dists[sl], dtype=np.float32),
                "z_vals": np.ascontiguousarray(z_vals[sl], dtype=np.float32),
                "rays_far": np.ascontiguousarray(rays_far[sl], dtype=np.float32),
            }
        )
    res = run_bass_kernel_spmd(nc, in_maps, core_ids=list(range(NCORES)))
    outs = res.results
    rgb_map = np.concatenate([outs[c]["rgb_map"] for c in range(NCORES)], axis=0)
    depth_map = np.concatenate([outs[c]["depth_map"] for c in range(NCORES)], axis=0)
    alpha = np.concatenate([outs[c]["alpha"] for c in range(NCORES)], axis=0)
    weights = np.concatenate([outs[c]["weights"] for c in range(NCORES)], axis=0)
    bg_weight = np.concatenate([outs[c]["bg_weight"] for c in range(NCORES)], axis=0)
    return rgb_map, depth_map, alpha, weights, bg_weight
